# revision 1
# baseline (speedup 1.0000x reference)
"""Distributed Trainium2 kernel for nn_AdjLoss (BCE between sigmoid Gram matrix
and sparse symmetric adjacency).

The float32 reference saturates: sigmoid(z) rounds to exactly 1.0 for
z >= T1 = 16.635532 (24*ln2), so log1p(-res) hits the -100 clamp and those
cells contribute exactly 100. Per-cell off-diagonal term (a = adjacency):
  a=0: T0(z) = softplus(z)   if z < T1, else 100
  a=1: T1(z) = softplus(-z)  and softplus(-z) - softplus(z) = -z exactly.

Approximations (rel-err budget 2e-2; measured 1.7e-3 end-to-end on the
real data, host-simulated):
  - softplus(z) ~= relu(z)  (z ~ N(0,256): error ln(1+e^-|z|) negligible)
  - fp8(e4m3) Gram matmul via DoubleRow perf mode (2 MACs/cell/cycle)
  - per-cell term min(relu(z),T1) + (100-T1)*[z>=T1]; the count comes from
    a steep ACT sigmoid on the bf16-rounded clamped values.

Work layout (fully static SPMD -- the per-core differences live in DATA):
  8192x8192 Gram upper-block-triangle = 544 tiles of 128x512 = (panel p,
  column-chunk q) with q >= p//4.  Column-chunk q holds 4q+4 tiles, so the
  chunk pair {r, 15-r} is exactly 68 tiles for every core r.  Each core
  processes 17 groups of 4 tiles; group g reads rhs window g of a
  host-packed per-core buffer (so all rhs offsets are static slot offsets,
  which the fp8 DoubleRow ISA checks require), and per-tile fp8 weight
  slabs at static positions.  Groups 0 and 1 are the two diagonal-block
  groups (chunk A=r rows 4r..4r+3, chunk B=15-r rows 60-4r..63-4r); the
  host applies the diag-block halving trick (the union of diag groups over
  cores is the 16 symmetric 512x512 diagonal blocks, and every true
  diagonal cell saturates: z_ii = ||l_i||^2 > T1, contributing exactly
  100).

Per group: 4 DoubleRow matmuls -> PSUM f32 [128,2048]; DVE
tensor_scalar(min T1, max 0) with fused row-sum accumulation (also the
PSUM->SBUF move, bf16 out); ACT sigmoid(1024*(scrA-16.6)) with fused
row-sum accumulation = saturation count.  Host sums 8x[128,34] partials,
adds the exact edge corrections (-z per unique smooth edge, -100 per
saturated edge, +100 per self-loop node).
"""

import sys

import numpy as np

if "/opt/trn_rl_repo" not in sys.path:
    sys.path.append("/opt/trn_rl_repo")

import concourse.bass as bass  # noqa: F401  (kept for parity with tooling)
import concourse.bacc as bacc
import concourse.mybir as mybir
from concourse.tile import TileContext

P = 128  # partitions
CT = 512  # column tile width
D = 256
KCH = D // P  # 2 contraction chunks
NCORES = 8
GW = 4 * CT  # group width
T1 = float(np.float32(16.635532))  # f32 sigmoid saturation threshold (24*ln2)
F_SAT = 100.0 - T1  # per-saturated-cell extra under the relu approximation
SIG_SCALE = 4096.0
SIG_CB = T1 - 0.003  # just below T1: saturated cells (scrA == T1) -> 1.0
USE_FP8 = True


class Cfg:
    def __init__(self, n):
        assert n == 8192
        self.N = n
        self.NQ = n // CT  # 16 column chunks
        self.NUNITS = 68
        self.NGROUPS = 17
        self.NDIAG_GROUPS = 2
        # canonical per-core layout: (panel, window-slot) per unit; the rhs
        # window content per slot is per-core data
        self.core_units = []  # [(panel, group)] in emission order
        self.core_windows = []  # chunk index backing each group slot
        for r in range(NCORES):
            a, b = r, 15 - r
            units = []
            windows = []
            # group 0: diag of chunk a; group 1: diag of chunk b
            units += [(4 * a + i, 0) for i in range(4)]
            windows.append(a)
            units += [(4 * b + i, 1) for i in range(4)]
            windows.append(b)
            g = 2
            for p0 in range(0, 4 * a, 4):  # chunk-a nondiag panels 0..4a-1
                units += [(p0 + i, g) for i in range(4)]
                windows.append(a)
                g += 1
            for p0 in range(0, 4 * b, 4):  # chunk-b nondiag panels 0..4b-1
                units += [(p0 + i, g) for i in range(4)]
                windows.append(b)
                g += 1
            assert g == self.NGROUPS and len(units) == self.NUNITS
            self.core_units.append(units)
            self.core_windows.append(windows)
        self.ACC_A0 = 0  # softplus sums
        self.ACC_B0 = self.NGROUPS  # saturation counts
        self.ACC_COLS = 2 * self.NGROUPS
        # count engine per group: 'a' ACT sigmoid, 'v' DVE is_ge (balance)
        self.countB = ["a"] * self.NGROUPS
        for g in (2, 4, 6, 8, 10, 12, 14):
            self.countB[g] = "v"


CFG_FULL = Cfg(8192)

BF16 = mybir.dt.bfloat16
F32 = mybir.dt.float32
FP8 = mybir.dt.float8e4
MMDT = FP8 if USE_FP8 else BF16


def build_kernel(cfg: Cfg) -> bass.Bass:
    nc = bacc.Bacc(None, target_bir_lowering=False, debug=False)

    NW = cfg.NGROUPS * CT  # packed rhs columns
    rhs_d = nc.declare_dram_parameter("rhs", [P, KCH, NW], MMDT, isOutput=False)
    lhs_d = nc.declare_dram_parameter(
        "lhs", [P, cfg.NUNITS, KCH, P], MMDT, isOutput=False
    )
    out_d = nc.declare_dram_parameter("out", [P, cfg.ACC_COLS], F32, isOutput=True)

    with TileContext(nc) as tc:
        with (
            tc.tile_pool(name="const", bufs=1) as cpool,
            tc.tile_pool(name="psum", bufs=2, space="PSUM") as ppool,
            tc.tile_pool(name="sa", bufs=4) as apool,
            tc.tile_pool(name="sb", bufs=3) as bpool,
        ):
            rhs = cpool.tile([P, KCH, NW], MMDT, tag="rhs")
            lhs = cpool.tile([P, cfg.NUNITS, KCH, P], MMDT, tag="lhs")
            # chunked input DMAs, interleaved so early groups unblock first
            bounds = [0, 4, 8, 12, 17]
            for ci in range(4):
                g0, g1 = bounds[ci], bounds[ci + 1]
                nc.sync.dma_start(
                    out=rhs[:, :, g0 * CT : g1 * CT],
                    in_=rhs_d[:, :, g0 * CT : g1 * CT],
                )
                nc.sync.dma_start(
                    out=lhs[:, 4 * g0 : 4 * g1, :, :],
                    in_=lhs_d[:, 4 * g0 : 4 * g1, :, :],
                )
            acc = cpool.tile([P, cfg.ACC_COLS], F32, tag="acc")
            nc.vector.memset(acc[:, :], 0.0)
            # scale/bias operands for the ACT sigmoid count
            sc_t = cpool.tile([P, 1], F32, tag="sc")
            nc.vector.memset(sc_t[:, :], SIG_SCALE)
            bi_t = cpool.tile([P, 1], F32, tag="bi")
            nc.vector.memset(bi_t[:, :], -SIG_SCALE * SIG_CB)

            scrAw = None
            for g in range(cfg.NGROUPS):
                psum_t = ppool.tile([P, GW], F32, tag="psum")
                for qi in range(4):
                    u = 4 * g + qi
                    if USE_FP8:
                        nc.tensor.matmul(
                            psum_t[:, qi * CT : (qi + 1) * CT],
                            lhs[:, u, :, :],
                            rhs[:, :, g * CT : (g + 1) * CT],
                            start=True,
                            stop=True,
                            perf_mode=mybir.MatmulPerfMode.DoubleRow,
                        )
                    else:
                        for k in range(KCH):
                            nc.tensor.matmul(
                                psum_t[:, qi * CT : (qi + 1) * CT],
                                lhs[:, u, k, :],
                                rhs[:, k, g * CT : (g + 1) * CT],
                                start=(k == 0),
                                stop=(k == KCH - 1),
                            )
                # PSUM->SBUF move with top clamp: scrA = min(z, T1).
                # NOTE the accum_out of tensor_scalar reduces with op1, so
                # only op1=add gives a row sum; the clamp keeps the later
                # passes single-breakpoint.  Movers for a PAIR of groups
                # share one double-width tile so the ACT relu-sum runs once
                # per pair (halves ACT op + accum-read + semaphore count).
                if g % 2 == 0:
                    pw = 2 * GW if g + 1 < cfg.NGROUPS else GW
                    scrAw = apool.tile([P, pw], F32, tag="scrA")
                half = (g % 2) * GW
                scrA = scrAw[:, half : half + GW]
                nc.vector.tensor_scalar(
                    scrA,
                    psum_t[:, :],
                    T1,
                    0.0,
                    mybir.AluOpType.min,
                    mybir.AluOpType.add,
                )
                # capped-relu sum: relu(scrA) = min(relu(z), T1), fused row
                # sums (Relu and Sigmoid share one activation table here)
                if g % 2 == 1 or g == cfg.NGROUPS - 1:
                    pj = g // 2
                    scrB = bpool.tile([P, pw], F32, tag="scrB")
                    nc.scalar.activation(
                        scrB[:, :],
                        scrAw[:, :pw],
                        mybir.ActivationFunctionType.Relu,
                        accum_out=acc[:, cfg.ACC_A0 + pj : cfg.ACC_A0 + pj + 1],
                    )
                # saturation count (scrA == T1 exactly iff z >= T1)
                bcol = acc[:, cfg.ACC_B0 + g : cfg.ACC_B0 + g + 1]
                if cfg.countB[g] == "a":
                    scrC = bpool.tile([P, GW], F32, tag="scrC")
                    nc.scalar.activation(
                        scrC[:, :],
                        scrA,
                        mybir.ActivationFunctionType.Sigmoid,
                        bias=bi_t[:, :],
                        scale=sc_t[:, :],
                        accum_out=bcol,
                    )
                else:
                    scrC = bpool.tile([P, GW], F32, tag="scrCv")
                    nc.vector.tensor_scalar(
                        scrC[:, :],
                        scrA,
                        T1,
                        0.0,
                        mybir.AluOpType.is_ge,
                        mybir.AluOpType.add,
                        accum_out=bcol,
                    )

            nc.sync.dma_start(out=out_d[:, :], in_=acc[:, :])

    if not nc.is_finalized():
        nc.finalize()
    return nc


def prep_inputs(l_enc: np.ndarray, edge_index: np.ndarray, cfg: Cfg):
    """Shard full inputs into 8 per-core input maps + host-side constants."""
    import ml_dtypes

    n, d = l_enc.shape
    assert n == cfg.N and d == D
    mdt = ml_dtypes.float8_e4m3fn if USE_FP8 else ml_dtypes.bfloat16
    lq = l_enc.astype(mdt)
    lT = np.ascontiguousarray(lq.T)  # [D, N]

    # edges: unique u<v pairs; self-loop node count; saturation class split
    u = np.asarray(edge_index[0], np.int64)
    v = np.asarray(edge_index[1], np.int64)
    n_self = len(np.unique(u[u == v]))
    a = np.minimum(u, v)
    b = np.maximum(u, v)
    nd = a != b
    keys = np.unique(a[nd] * n + b[nd])
    ua = (keys // n).astype(np.int64)
    ub = (keys % n).astype(np.int64)
    # the diag-block halving trick requires every true-diagonal cell to be
    # saturated (z_ii = ||l_i||^2 >= T1) in the quantized matmul
    lqf = lq.astype(np.float32)
    assert float((lqf * lqf).sum(1).min()) > T1 + 1.0
    # classify: edges whose f32 Gram value saturates the f32 sigmoid
    ze = np.einsum("ij,ij->i", l_enc[ua], l_enc[ub]).astype(np.float32)
    sat = ze >= np.float32(T1)
    n_sat_edges = int(sat.sum())
    ua, ub = ua[~sat], ub[~sat]
    # exact smooth-edge correction: softplus(-z) - softplus(z) = -z
    smooth_edge_sum = float(
        np.einsum("ij,ij->", l_enc[ua].astype(np.float64), l_enc[ub].astype(np.float64))
    )

    NW = cfg.NGROUPS * CT
    in_maps = []
    for r in range(NCORES):
        rhs_np = np.zeros((P, KCH, NW), mdt)
        for g, w in enumerate(cfg.core_windows[r]):
            for k in range(KCH):
                rhs_np[:, k, g * CT : (g + 1) * CT] = lT[
                    k * P : (k + 1) * P, w * CT : (w + 1) * CT
                ]
        lhs_np = np.zeros((P, cfg.NUNITS, KCH, P), mdt)
        for uu, (p, _) in enumerate(cfg.core_units[r]):
            for k in range(KCH):
                lhs_np[:, uu, k, :] = lT[k * P : (k + 1) * P, p * P : (p + 1) * P]
        in_maps.append({"rhs": rhs_np, "lhs": lhs_np})
    return in_maps, n_self, n_sat_edges, smooth_edge_sum


def combine(results, n_self, n_sat_edges, cfg, host_edge_sum):
    acc = np.zeros(cfg.ACC_COLS, np.float64)
    for i in range(NCORES):
        acc += results[i]["out"].astype(np.float64).sum(0)
    npairs = (cfg.NGROUPS + 1) // 2
    a_sums = acc[cfg.ACC_A0 : cfg.ACC_A0 + npairs]  # per group-PAIR
    cnts = acc[cfg.ACC_B0 : cfg.ACC_B0 + cfg.NGROUPS]
    ndg = cfg.NDIAG_GROUPS  # == 2 == exactly pair 0
    t0_diag = a_sums[0] + F_SAT * cnts[:ndg].sum()
    t0_rest = a_sums[1:].sum() + F_SAT * cnts[ndg:].sum()
    # diag blocks: total = 2*(strict upper) + N*100 (every true-diagonal
    # cell contributes T1 + FP_SAT = 100 exactly)
    u_tri = (t0_diag - 100.0 * cfg.N) / 2.0 + t0_rest
    total = u_tri - host_edge_sum - 100.0 * n_sat_edges
    return np.float32((2.0 * total + 100.0 * n_self) / float(cfg.N) ** 2)


_COMPILED = {}


def kernel(l_enc: np.ndarray, edge_index: np.ndarray) -> np.ndarray:
    from concourse.bass_utils import run_bass_kernel_spmd

    cfg = CFG_FULL
    l_enc = np.asarray(l_enc, np.float32)
    in_maps, n_self, n_sat_edges, hes = prep_inputs(
        l_enc, np.asarray(edge_index), cfg
    )
    if "full" not in _COMPILED:
        _COMPILED["full"] = build_kernel(cfg)
    nc = _COMPILED["full"]
    res = run_bass_kernel_spmd(nc, in_maps, core_ids=list(range(NCORES)))
    return combine(res.results, n_self, n_sat_edges, cfg, hes)

